# revision 4
# baseline (speedup 1.0000x reference)
"""AttentionBlock (GroupNorm + single-head self-attention + residual) on 8 TRN2 cores.

Sharding: 8 cores = 4 batch samples x 2 query-halves. Each core receives the
full 4096-token sample with its own half's tokens permuted to the front
(GroupNorm stats, K/V and softmax sums are token-permutation invariant), computes
GroupNorm + QKV + attention for its 2048 query rows, and writes [2048, 256].

Per-core pipeline:
  B) load x token-major, GroupNorm group sums via DVE reduces + ones-matmul
  C) PE-transpose x to channel-major, fusing (x*s + t) normalize into the
     PSUM->SBUF copy on the scalar engine (per-partition scale/bias)
  D) QKV projections (fp32r matmuls), V kept token-major for the PV matmul
  E) flash-style attention over 512-query blocks: S^T = K^T Q per 128-key tile,
     exp on scalar engine (no max subtraction: |S|<~6 for these inputs),
     PV accumulated in PSUM over all 32 key tiles, softmax denominator
     accumulated on DVE and divided out only after the output projection
  F) output projection + denominator normalize + residual + store
"""

import numpy as np
from contextlib import ExitStack

import concourse.bass as bass
import concourse.bacc as bacc
import concourse.tile as tile
from concourse import mybir
from concourse.bass_utils import run_bass_kernel_spmd
from concourse.masks import make_identity

F32 = mybir.dt.float32
F32R = mybir.dt.float32r
AX = mybir.AxisListType.X
AF = mybir.ActivationFunctionType

B, H, W, C = 4, 64, 64, 256
TOK = H * W          # 4096 tokens per sample
NQ = TOK // 2        # 2048 query rows per core
G, GS = 8, C // 8    # groups, group size
EPS = 1e-3
SCALE = float(C) ** -0.5
N_CORES = 8
NT = TOK // 128      # 32 token tiles
NQT = NQ // 128      # 16 query token tiles
NB = NQ // 512       # 4 query blocks
CT = C // 128        # 2 channel tiles


def build_nc(use_f32r=True):
    mmdt = F32R if use_f32r else F32
    nc = bacc.Bacc(trn_type="TRN2")

    xs_d = nc.declare_dram_parameter("xs", [TOK, C], F32, isOutput=False)
    wq_d = nc.declare_dram_parameter("Wq", [C, C], mmdt, isOutput=False)
    wk_d = nc.declare_dram_parameter("Wk", [C, C], mmdt, isOutput=False)
    wv_d = nc.declare_dram_parameter("Wv", [C, C], mmdt, isOutput=False)
    wp_d = nc.declare_dram_parameter("Wp", [C, C], mmdt, isOutput=False)
    bq_d = nc.declare_dram_parameter("bq", [C], F32, isOutput=False)
    bk_d = nc.declare_dram_parameter("bk", [C], F32, isOutput=False)
    bv_d = nc.declare_dram_parameter("bv", [C], mmdt, isOutput=False)
    bp_d = nc.declare_dram_parameter("bp", [C], F32, isOutput=False)
    gam_d = nc.declare_dram_parameter("gn_gamma", [C], F32, isOutput=False)
    bet_d = nc.declare_dram_parameter("gn_beta", [C], F32, isOutput=False)
    out_d = nc.declare_dram_parameter("out", [NQ, C], F32, isOutput=True)

    with tile.TileContext(nc) as tc, ExitStack() as stack:
        consts = stack.enter_context(tc.tile_pool(name="consts", bufs=1))
        persist = stack.enter_context(tc.tile_pool(name="persist", bufs=1))
        dram = stack.enter_context(tc.tile_pool(name="dram", bufs=1, space="DRAM"))

        # ---- Phase A: constants ----
        ident = consts.tile([128, 128], F32)
        make_identity(nc, ident)
        ones = consts.tile([128, 1], F32)
        nc.vector.memset(ones, 1.0)
        epsc = consts.tile([1, 1], F32)
        nc.vector.memset(epsc, EPS)

        grow = consts.tile([1, C], F32)
        nc.sync.dma_start(out=grow, in_=gam_d[:].rearrange("(a c) -> a c", a=1))
        brow = consts.tile([1, C], F32)
        nc.sync.dma_start(out=brow, in_=bet_d[:].rearrange("(a c) -> a c", a=1))
        bprow = consts.tile([1, C], F32)
        nc.sync.dma_start(out=bprow, in_=bp_d[:].rearrange("(a c) -> a c", a=1))

        wq_t, wk_t, wv_t, wp_t = [], [], [], []
        for kk in range(CT):
            for lst, src, nm in (
                (wq_t, wq_d, "wq"), (wk_t, wk_d, "wk"),
                (wv_t, wv_d, "wv"), (wp_t, wp_d, "wp"),
            ):
                t = consts.tile([128, C], mmdt, name=f"{nm}{kk}")
                nc.sync.dma_start(out=t, in_=src[kk * 128:(kk + 1) * 128, :])
                lst.append(t)
        bqc, bkc, bvc = [], [], []
        for m in range(CT):
            tq = consts.tile([128, 1], F32, name=f"bqc{m}")
            nc.sync.dma_start(
                out=tq, in_=bq_d[m * 128:(m + 1) * 128].rearrange("(p a) -> p a", a=1))
            bqc.append(tq)
            tk = consts.tile([128, 1], F32, name=f"bkc{m}")
            nc.sync.dma_start(
                out=tk, in_=bk_d[m * 128:(m + 1) * 128].rearrange("(p a) -> p a", a=1))
            bkc.append(tk)
            tv = consts.tile([128, 1], mmdt, name=f"bvc{m}")
            nc.sync.dma_start(
                out=tv, in_=bv_d[m * 128:(m + 1) * 128].rearrange("(p a) -> p a", a=1))
            bvc.append(tv)

        # ---- Phase B: load x + GroupNorm statistics ----
        xk = [persist.tile([128, C], F32, name=f"xk{i}") for i in range(NQT)]
        statp = stack.enter_context(tc.tile_pool(name="statp", bufs=1))
        partials = statp.tile([128, NT, 2 * G], F32)
        xk += [persist.tile([128, C], F32, name=f"xk{i}") for i in range(NQT, NT)]
        xt = []
        with (
            tc.tile_pool(name="sqp", bufs=3) as sqp,
            tc.tile_pool(name="statps", bufs=1, space="PSUM") as statps,
        ):
            for i in range(NT):
                t = xk[i]
                nc.sync.dma_start(out=t, in_=xs_d[i * 128:(i + 1) * 128, :])
                xt.append(t)
                sqt = sqp.tile([128, C], F32, tag="sq")
                nc.scalar.activation(sqt, t, AF.Square)
                nc.vector.reduce_sum(
                    out=partials[:, i, 0:G],
                    in_=t.rearrange("p (g d) -> p g d", g=G), axis=AX)
                nc.vector.reduce_sum(
                    out=partials[:, i, G:2 * G],
                    in_=sqt.rearrange("p (g d) -> p g d", g=G), axis=AX)
            totals = statp.tile([128, 2 * G], F32)
            nc.vector.reduce_sum(
                out=totals, in_=partials.rearrange("p a b -> p b a"), axis=AX)
            stats_ps = statps.tile([1, 2 * G], F32, tag="st16")
            nc.tensor.matmul(stats_ps, ones, totals, start=True, stop=True)
            srow16 = statp.tile([1, 2 * G], F32)
            nc.scalar.copy(srow16, stats_ps)

            # group math: g16 = [rstd_g | mean_g]
            g16 = statp.tile([1, 2 * G], F32)
            meang = g16[:, G:2 * G]
            nc.scalar.mul(meang, srow16[:, 0:G], 1.0 / (TOK * GS))
            msqg = statp.tile([1, G], F32)
            nc.scalar.mul(msqg, srow16[:, G:2 * G], 1.0 / (TOK * GS))
            m2 = statp.tile([1, G], F32)
            nc.vector.tensor_mul(m2, meang, meang)
            varg = statp.tile([1, G], F32)
            nc.vector.tensor_sub(varg, msqg, m2)
            stdg = statp.tile([1, G], F32)
            nc.scalar.activation(stdg, varg, AF.Sqrt, bias=epsc, scale=1.0)
            nc.vector.reciprocal(g16[:, 0:G], stdg)

            # expand groups -> channels: step-0 broadcast reads on DVE
            rstd_b = statp.tile([1, C], F32)
            nc.vector.tensor_copy(
                rstd_b.rearrange("a (g d) -> a g d", g=G),
                g16[:, 0:G].rearrange("a (g d) -> a g d", g=G).to_broadcast((1, G, GS)))
            mean_b = statp.tile([1, C], F32)
            nc.vector.tensor_copy(
                mean_b.rearrange("a (g d) -> a g d", g=G),
                g16[:, G:2 * G].rearrange("a (g d) -> a g d", g=G).to_broadcast((1, G, GS)))

            # per-channel scale s and shift t rows
            srow = statp.tile([1, C], F32)
            nc.vector.tensor_mul(srow, rstd_b, grow)
            tmpr = statp.tile([1, C], F32)
            nc.vector.tensor_mul(tmpr, mean_b, srow)
            trow = statp.tile([1, C], F32)
            nc.vector.tensor_sub(trow, brow, tmpr)

            # bv @ Wp folded into the final bias (P rows sum to 1)
            bvwp_ps = statps.tile([1, C], F32, tag="bvwp")
            for kk in range(CT):
                nc.tensor.matmul(bvwp_ps, bvc[kk], wp_t[kk],
                                 start=(kk == 0), stop=(kk == CT - 1))
            tfin = statp.tile([1, C], F32)
            nc.scalar.copy(tfin, bvwp_ps)
            nc.vector.tensor_add(tfin, tfin, trow)
            nc.vector.tensor_add(tfin, tfin, bprow)

            # scatter s/t to DRAM, reload as columns and broadcasts
            sscr = dram.tile([C], F32)
            nc.sync.dma_start(out=sscr, in_=srow)
            tscr = dram.tile([C], F32)
            nc.sync.dma_start(out=tscr, in_=trow)
            tfscr = dram.tile([C], F32)
            nc.sync.dma_start(out=tfscr, in_=tfin)

            scol, tcol = [], []
            for cc in range(CT):
                sc = persist.tile([128, 1], F32, name=f"scol{cc}")
                nc.gpsimd.dma_start(
                    out=sc, in_=bass.AP(tensor=sscr.tensor,
                                        offset=sscr.offset + cc * 128, ap=[[1, 128]]))
                scol.append(sc)
                tc_ = persist.tile([128, 1], F32, name=f"tcol{cc}")
                nc.gpsimd.dma_start(
                    out=tc_, in_=bass.AP(tensor=tscr.tensor,
                                         offset=tscr.offset + cc * 128, ap=[[1, 128]]))
                tcol.append(tc_)
            s_bcast = persist.tile([128, C], F32)
            nc.gpsimd.dma_start(
                out=s_bcast, in_=bass.AP(tensor=sscr.tensor, offset=sscr.offset,
                                         ap=[[0, 128], [1, C]]))
            tf_bcast = persist.tile([128, C], F32)
            nc.gpsimd.dma_start(
                out=tf_bcast, in_=bass.AP(tensor=tfscr.tensor, offset=tfscr.offset,
                                          ap=[[0, 128], [1, C]]))

            # ---- Phase C: transpose + normalize -> xn channel-major ----
            xn_cm = [persist.tile([128, TOK], mmdt, name=f"xncm{cc}")
                     for cc in range(CT)]
            with tc.tile_pool(name="tps", bufs=4, space="PSUM") as tps:
                for i in range(NT):
                    for cc in range(CT):
                        tp = tps.tile([128, 128], F32, tag="tp")
                        nc.tensor.transpose(
                            tp, xt[i][:, cc * 128:(cc + 1) * 128], ident)
                        nc.scalar.activation(
                            out=xn_cm[cc][:, i * 128:(i + 1) * 128], in_=tp,
                            func=AF.Identity, bias=tcol[cc], scale=scol[cc])

        # ---- Phase D: QKV projections ----
        k_cm = [persist.tile([128, TOK], mmdt, name=f"kcm{m}") for m in range(CT)]
        q_cm = [persist.tile([128, NQ], mmdt, name=f"qcm{m}") for m in range(CT)]
        v_t = [persist.tile([128, C], mmdt, name=f"v{t}") for t in range(NT)]
        with tc.tile_pool(name="qkps", bufs=4, space="PSUM") as qkps:
            for m in range(CT):
                for blk in range(NB):
                    qp = qkps.tile([128, 512], F32, tag="qk")
                    for kk in range(CT):
                        nc.tensor.matmul(
                            qp, wq_t[kk][:, m * 128:(m + 1) * 128],
                            xn_cm[kk][:, blk * 512:(blk + 1) * 512],
                            start=(kk == 0), stop=(kk == CT - 1))
                    nc.scalar.activation(
                        out=q_cm[m][:, blk * 512:(blk + 1) * 512], in_=qp,
                        func=AF.Identity, bias=bqc[m], scale=1.0)
            for m in range(CT):
                for blk in range(TOK // 512):
                    kp = qkps.tile([128, 512], F32, tag="qk")
                    for kk in range(CT):
                        nc.tensor.matmul(
                            kp, wk_t[kk][:, m * 128:(m + 1) * 128],
                            xn_cm[kk][:, blk * 512:(blk + 1) * 512],
                            start=(kk == 0), stop=(kk == CT - 1))
                    nc.scalar.activation(
                        out=k_cm[m][:, blk * 512:(blk + 1) * 512], in_=kp,
                        func=AF.Identity, bias=bkc[m], scale=1.0)
            for t in range(NT):
                vp = qkps.tile([128, C], F32, tag="vps", bufs=3)
                for kk in range(CT):
                    nc.tensor.matmul(
                        vp, xn_cm[kk][:, t * 128:(t + 1) * 128], wv_t[kk],
                        start=(kk == 0), stop=(kk == CT - 1))
                nc.scalar.copy(v_t[t], vp)

        # residual xn rows (token-major), in place over the kept x tiles
        for t in range(NQT):
            nc.vector.tensor_mul(xk[t], xk[t], s_bcast)
            nc.vector.tensor_add(xk[t], xk[t], tf_bcast)

        # ---- Phase E: attention ----
        ev_sb = [persist.tile([128, NQ], mmdt, name=f"evsb{cc}") for cc in range(CT)]
        dinv = persist.tile([128, NQT], F32)
        dcol = persist.tile([128, NQT], F32)
        dscr = [dram.tile([512], F32, name=f"dscr{nb}") for nb in range(NB)]
        with (
            tc.tile_pool(name="etp", bufs=3) as etp,
            tc.tile_pool(name="accp", bufs=2) as accp,
            tc.tile_pool(name="drp", bufs=2) as drp,
            tc.tile_pool(name="stp", bufs=3, space="PSUM") as stp,
            tc.tile_pool(name="evp", bufs=1, space="PSUM") as evp,
            tc.tile_pool(name="dp", bufs=2, space="PSUM") as dp,
        ):
            for nb in range(NB):
                ev0 = evp.tile([128, 512], F32, tag="ev0")
                ev1 = evp.tile([128, 512], F32, tag="ev1")
                accd = accp.tile([128, 512], F32, tag="acc")
                for mt in range(NT):
                    st = stp.tile([128, 512], F32, tag="st")
                    for kk in range(CT):
                        nc.tensor.matmul(
                            st, k_cm[kk][:, mt * 128:(mt + 1) * 128],
                            q_cm[kk][:, nb * 512:(nb + 1) * 512],
                            start=(kk == 0), stop=(kk == CT - 1))
                    et = etp.tile([128, 512], mmdt, tag="et")
                    nc.scalar.activation(et, st, AF.Exp, scale=SCALE)
                    etf = et.bitcast(F32)
                    if mt == 0:
                        nc.vector.tensor_copy(accd, etf)
                    else:
                        nc.vector.tensor_add(accd, accd, etf)
                    nc.tensor.matmul(ev0, v_t[mt][:, 0:128], et,
                                     start=(mt == 0), stop=(mt == NT - 1))
                    nc.tensor.matmul(ev1, v_t[mt][:, 128:C], et,
                                     start=(mt == 0), stop=(mt == NT - 1))
                dps = dp.tile([1, 512], F32, tag="d")
                nc.tensor.matmul(dps, ones, accd, start=True, stop=True)
                drowt = drp.tile([1, 512], F32, tag="dr")
                nc.scalar.copy(drowt, dps)
                nc.sync.dma_start(out=dscr[nb], in_=drowt)
                nc.gpsimd.dma_start(
                    out=dcol[:, nb * 4:(nb + 1) * 4],
                    in_=bass.AP(tensor=dscr[nb].tensor, offset=dscr[nb].offset,
                                ap=[[1, 128], [128, 4], [1, 1]]))
                nc.vector.reciprocal(dinv[:, nb * 4:(nb + 1) * 4],
                                     dcol[:, nb * 4:(nb + 1) * 4])
                nc.scalar.copy(ev_sb[0][:, nb * 512:(nb + 1) * 512], ev0)
                nc.scalar.copy(ev_sb[1][:, nb * 512:(nb + 1) * 512], ev1)

        # ---- Phase F: output projection + normalize + residual ----
        with (
            tc.tile_pool(name="outp", bufs=3) as outp,
            tc.tile_pool(name="yps", bufs=3, space="PSUM") as yps,
        ):
            for t in range(NQT):
                yp = yps.tile([128, C], F32, tag="y")
                for kk in range(CT):
                    nc.tensor.matmul(
                        yp, ev_sb[kk][:, t * 128:(t + 1) * 128], wp_t[kk],
                        start=(kk == 0), stop=(kk == CT - 1))
                yn = outp.tile([128, C], F32, tag="yn")
                nc.scalar.activation(yn, yp, AF.Copy, scale=dinv[:, t:t + 1])
                ot = outp.tile([128, C], F32, tag="ot")
                nc.vector.tensor_add(ot, yn, xk[t])
                nc.sync.dma_start(out=out_d[t * 128:(t + 1) * 128, :], in_=ot)

    nc.finalize()
    return nc


_NC_CACHE = {}


def _get_nc(use_f32r=True):
    if use_f32r not in _NC_CACHE:
        _NC_CACHE[use_f32r] = build_nc(use_f32r)
    return _NC_CACHE[use_f32r]


def run(inputs, use_f32r=True, trace=False):
    x = np.ascontiguousarray(np.asarray(inputs["x"], np.float32)).reshape(B, TOK, C)
    common = {
        k: np.ascontiguousarray(np.asarray(inputs[k], np.float32))
        for k in ["Wq", "Wk", "Wv", "Wp", "bq", "bk", "bv", "bp",
                  "gn_gamma", "gn_beta"]
    }
    in_maps = []
    for core in range(N_CORES):
        b, h = core // 2, core % 2
        if h == 0:
            xs = x[b]
        else:
            xs = np.concatenate([x[b][NQ:], x[b][:NQ]], axis=0)
        in_maps.append({"xs": np.ascontiguousarray(xs), **common})

    nc = _get_nc(use_f32r)
    res = run_bass_kernel_spmd(nc, in_maps, list(range(N_CORES)), trace=trace)

    out = np.empty((B, TOK, C), np.float32)
    for core in range(N_CORES):
        b, h = core // 2, core % 2
        out[b, h * NQ:(h + 1) * NQ] = res.results[core]["out"]
    return out.reshape(B, H, W, C), res


def kernel(**inputs):
    out, _ = run(inputs)
    return out
